# revision 4
# baseline (speedup 1.0000x reference)
"""Trainium2 Bass kernel for nn_CrossEntropy_54322746360602.

reference:
    dots = A @ B.T                                  [8192, 8192]
    cos  = dots / max(|a_i| * |b_j|, 1e-8)
    logp = log_softmax(cos, axis=1)
    mask = (label_i == label_j)
    per_row = -(mask * logp).sum(1) / mask.sum(1)
    loss = per_row.mean()
    returns (loss, cos)

Strategy (8 NeuronCores, SPMD, no collectives):
  - Shard rows of A: 1024 rows per core.  Each core computes its
    [1024, 8192] cos slab + per-row stats; host concatenates slabs and
    averages per-row losses.
  - Host does LAYOUT-only prep (transposes / index reshapes); all FLOPs
    (norms, matmul, softmax sums, masked sums) run on device.
  - Device per core:
      * A natural [1024,1024] f32 -> row sumsq (fused DVE mult+reduce)
        -> inv_na [128,8] (sqrt + reciprocal).
      * B^T f32 loaded in 4 column groups of 2048, cast to bf16 during
        DMA (SWDGE cast).  Column sumsq via ones-vector matmul on PE,
        inv_nb broadcast to [128, 2048] via a DRAM round trip.
      * Main loop: dots tile [128i, 512j] = sum_d A_bf^T_d.T @ B_bf_d
        in PSUM (bf16 matmul, f32 accumulate).
        cos = (dots * inv_na[i]) * inv_nb[j]   -- one fused DVE op
        s_part[i,j] = sum_j exp(cos)           -- ACT exp with accum_out
        spc_part[i,j] = sum_j (lab_j==lab_i)*cos  -- one fused DVE op
        DMA cos tile to the output slab.
      * Epilogue: per_row = log(sum_j exp) - spc/npos.  (Row max is not
        needed: cos in [-1, 1], exp can't overflow.  eps=1e-8 never
        binds: norms of N(0,1) rows of length 1024 are ~32.)
"""

import numpy as np
import ml_dtypes

import concourse.bass as bass
import concourse.mybir as mybir
import concourse.tile as tile
from concourse import bacc

F32 = mybir.dt.float32
BF16 = mybir.dt.bfloat16
AF = mybir.ActivationFunctionType
ALU = mybir.AluOpType
AX = mybir.AxisListType

N_CORES = 8
B_FULL = 8192
D_FULL = 1024


def build_nc(rows=1024, d=1024, bcols=8192, jgroup=2048, nblk=512):
    """Build the per-core Bass module (same NEFF on all cores)."""
    n_ib = rows // 128          # i-blocks (output partitions groups)
    n_dc = d // 128             # contraction chunks
    n_g = bcols // jgroup       # B column groups
    n_bpg = jgroup // nblk      # 512-wide j blocks per group
    n_jb = bcols // nblk        # total j blocks

    nc = bacc.Bacc("TRN2", target_bir_lowering=False, debug=False,
                   num_devices=N_CORES)

    a_t = nc.dram_tensor("a_t", [d, rows], F32, kind="ExternalInput")
    a_nat = nc.dram_tensor("a_nat", [rows, d], F32, kind="ExternalInput")
    b_t = nc.dram_tensor("b_t", [d, bcols], F32, kind="ExternalInput")
    labels_bf = nc.dram_tensor("labels_bf", [bcols], BF16, kind="ExternalInput")
    labi = nc.dram_tensor("labi", [128, n_ib], BF16, kind="ExternalInput")
    inv_npos = nc.dram_tensor("inv_npos", [128, n_ib], F32, kind="ExternalInput")
    cos_out = nc.dram_tensor("cos_out", [rows, bcols], F32, kind="ExternalOutput")
    per_row_out = nc.dram_tensor("per_row_out", [128, n_ib], F32,
                                 kind="ExternalOutput")

    with tile.TileContext(nc) as tc:
        with (
            tc.tile_pool(name="persist", bufs=1) as persist,
            tc.tile_pool(name="bgrp", bufs=2) as bgrp,
            tc.tile_pool(name="anat", bufs=2) as anat_pool,
            tc.tile_pool(name="sqp", bufs=2) as sqp,
            tc.tile_pool(name="invrow", bufs=2) as invrow_pool,
            tc.tile_pool(name="cosp", bufs=6) as cosp,
            tc.tile_pool(name="junk", bufs=2) as junkp,
            tc.tile_pool(name="psum", bufs=4, space="PSUM") as psum_pool,
            tc.tile_pool(name="psnb", bufs=1, space="PSUM") as psnb_pool,
            tc.tile_pool(name="dram", bufs=1, space="DRAM") as dramp,
        ):
            # ---------------- persistent tiles ----------------
            a_bf = [persist.tile([128, rows], BF16, tag=f"abf{dd}", name=f"abf{dd}")
                    for dd in range(n_dc)]
            labj = persist.tile([128, bcols], BF16, tag="labj", name="labj")
            labi_sb = persist.tile([128, n_ib], BF16, tag="labi", name="labi")
            invnpos_sb = persist.tile([128, n_ib], F32, tag="invnpos", name="invnpos")
            inv_na = persist.tile([128, n_ib], F32, tag="invna", name="invna")
            na2 = persist.tile([128, n_ib], F32, tag="na2", name="na2")
            ones_bf = persist.tile([128, 1], BF16, tag="ones", name="ones")
            s_parts = [persist.tile([128, n_jb], F32, tag=f"sp{i}", name=f"sp{i}")
                       for i in range(n_ib)]
            spc_parts = [persist.tile([128, n_jb], F32, tag=f"cp{i}", name=f"cp{i}")
                         for i in range(n_ib)]
            per_row = persist.tile([128, n_ib], F32, tag="perrow", name="perrow")
            invnb_bc = [persist.tile([128, jgroup], F32, tag=f"invnb{g}", name=f"invnb{g}")
                        for g in range(n_g)]
            invnb_scr = dramp.tile([bcols], F32, tag="invnb_scr", name="invnb_scr")

            nc.vector.memset(ones_bf, 1.0)

            # ---------------- small input loads ----------------
            nc.sync.dma_start(out=labi_sb, in_=labi.ap())
            nc.sync.dma_start(out=invnpos_sb, in_=inv_npos.ap())
            # broadcast labels over all 128 partitions
            lab_src = bass.AP(tensor=labels_bf, offset=0,
                              ap=[[0, 128], [1, bcols]])
            nc.gpsimd.dma_start(out=labj, in_=lab_src)

            # ---------------- A norms (natural layout, exact f32) -------
            for ib in range(n_ib):
                a_row = anat_pool.tile([128, d], F32)
                nc.gpsimd.dma_start(out=a_row,
                                    in_=a_nat[ib * 128:(ib + 1) * 128, :])
                tjunk = anat_pool.tile([128, d], F32, tag="ttrjunk", name="ttrjunk")
                nc.vector.scalar_tensor_tensor(
                    out=tjunk, in0=a_row, scalar=1.0, in1=a_row,
                    op0=ALU.mult, op1=ALU.mult,
                    accum_out=na2[:, ib:ib + 1])
            nc.scalar.activation(out=inv_na, in_=na2, func=AF.Sqrt)
            nc.vector.reciprocal(out=inv_na, in_=inv_na)

            # ---------------- A^T cast load (matmul lhsT) ---------------
            for dd in range(n_dc):
                nc.gpsimd.dma_start(out=a_bf[dd],
                                    in_=a_t[dd * 128:(dd + 1) * 128, :])

            # ---------------- per-group B processing + main loop --------
            for g in range(n_g):
                c0 = g * jgroup
                # cast-load this group's B^T chunks  (f32 -> bf16 in DMA)
                b_bf = [bgrp.tile([128, jgroup], BF16, tag=f"bg{dd}", name=f"bg{dd}")
                        for dd in range(n_dc)]
                for dd in range(n_dc):
                    nc.gpsimd.dma_start(
                        out=b_bf[dd],
                        in_=b_t[dd * 128:(dd + 1) * 128, c0:c0 + jgroup])

                # column sumsq via ones-matmul, accumulated over d chunks
                nb_ps = [psnb_pool.tile([1, nblk], F32, tag=f"nb{n}", name=f"nb{n}")
                         for n in range(n_bpg)]
                for dd in range(n_dc):
                    sq = sqp.tile([128, jgroup], BF16, tag="sq", name="sq")
                    nc.vector.tensor_mul(sq, b_bf[dd], b_bf[dd])
                    for n in range(n_bpg):
                        nc.tensor.matmul(
                            nb_ps[n], ones_bf, sq[:, n * nblk:(n + 1) * nblk],
                            start=(dd == 0), stop=(dd == n_dc - 1))
                inv_row = invrow_pool.tile([1, jgroup], F32, tag="invrow", name="invrow")
                for n in range(n_bpg):
                    nc.scalar.copy(inv_row[:, n * nblk:(n + 1) * nblk],
                                   nb_ps[n])
                nc.scalar.activation(out=inv_row, in_=inv_row, func=AF.Sqrt)
                nc.vector.reciprocal(out=inv_row, in_=inv_row)
                # broadcast to 128 partitions via DRAM round trip
                nc.sync.dma_start(out=invnb_scr[c0:c0 + jgroup], in_=inv_row)
                scr_src = bass.AP(tensor=invnb_scr.tensor,
                                  offset=invnb_scr.offset + c0,
                                  ap=[[0, 128], [1, jgroup]])
                nc.gpsimd.dma_start(out=invnb_bc[g], in_=scr_src)

                # ---------------- main loop for this group --------------
                for jj in range(n_bpg):
                    jb = g * n_bpg + jj
                    jsl = slice(jj * nblk, (jj + 1) * nblk)
                    for i in range(n_ib):
                        ps = psum_pool.tile([128, nblk], F32, tag="dots", name="dots")
                        for dd in range(n_dc):
                            nc.tensor.matmul(
                                ps, a_bf[dd][:, i * 128:(i + 1) * 128],
                                b_bf[dd][:, jsl],
                                start=(dd == 0), stop=(dd == n_dc - 1))
                        cos_sb = cosp.tile([128, nblk], F32, tag="cos", name="cos")
                        nc.vector.scalar_tensor_tensor(
                            out=cos_sb, in0=ps, scalar=inv_na[:, i:i + 1],
                            in1=invnb_bc[g][:, jsl],
                            op0=ALU.mult, op1=ALU.mult)
                        je = junkp.tile([128, nblk], BF16, tag="je", name="je")
                        nc.scalar.activation(
                            out=je, in_=cos_sb, func=AF.Exp,
                            accum_out=s_parts[i][:, jb:jb + 1])
                        jm = junkp.tile([128, nblk], BF16, tag="jm", name="jm")
                        nc.vector.scalar_tensor_tensor(
                            out=jm, in0=labj[:, jb * nblk:(jb + 1) * nblk],
                            scalar=labi_sb[:, i:i + 1], in1=cos_sb,
                            op0=ALU.is_equal, op1=ALU.mult,
                            accum_out=spc_parts[i][:, jb:jb + 1])
                        nc.sync.dma_start(
                            out=cos_out[i * 128:(i + 1) * 128,
                                        jb * nblk:(jb + 1) * nblk],
                            in_=cos_sb)

            # ---------------- epilogue: per-row loss --------------------
            for i in range(n_ib):
                s_red = cosp.tile([128, 1], F32, tag="sred", name="sred")
                nc.vector.reduce_sum(s_red, s_parts[i], axis=AX.X)
                nc.scalar.activation(out=s_red, in_=s_red, func=AF.Ln)
                c_red = cosp.tile([128, 1], F32, tag="cred", name="cred")
                nc.vector.reduce_sum(c_red, spc_parts[i], axis=AX.X)
                nc.vector.tensor_mul(c_red, c_red, invnpos_sb[:, i:i + 1])
                nc.vector.tensor_sub(per_row[:, i:i + 1], s_red, c_red)
            nc.sync.dma_start(out=per_row_out.ap(), in_=per_row)

    nc.compile()
    return nc


_NC_CACHE = {}


def get_nc():
    if "nc" not in _NC_CACHE:
        _NC_CACHE["nc"] = build_nc()
    return _NC_CACHE["nc"]


def make_in_maps(label_tensor, feature_a, feature_b):
    labels = np.asarray(label_tensor).astype(np.int64)
    A = np.ascontiguousarray(np.asarray(feature_a, dtype=np.float32))
    B = np.ascontiguousarray(np.asarray(feature_b, dtype=np.float32))
    b_t = np.ascontiguousarray(B.T)                       # [1024, 8192]
    counts = np.bincount(labels, minlength=1)
    inv_npos_full = (1.0 / counts[labels].astype(np.float64)).astype(np.float32)
    labels_bf = labels.astype(ml_dtypes.bfloat16)
    rows = B_FULL // N_CORES
    n_ib = rows // 128
    in_maps = []
    for c in range(N_CORES):
        sl = slice(c * rows, (c + 1) * rows)
        a_shard = A[sl]
        in_maps.append({
            "a_t": np.ascontiguousarray(a_shard.T),
            "a_nat": a_shard,
            "b_t": b_t,
            "labels_bf": labels_bf,
            "labi": np.ascontiguousarray(
                labels_bf[sl].reshape(n_ib, 128).T),
            "inv_npos": np.ascontiguousarray(
                inv_npos_full[sl].reshape(n_ib, 128).T),
        })
    return in_maps


def kernel(label_tensor, feature_a, feature_b):
    from concourse.bass_utils import run_bass_kernel_spmd

    in_maps = make_in_maps(label_tensor, feature_a, feature_b)
    nc = get_nc()
    res = run_bass_kernel_spmd(nc, in_maps, core_ids=list(range(N_CORES)))
    cos = np.concatenate([r["cos_out"] for r in res.results], axis=0)
    per_row = np.concatenate(
        [r["per_row_out"].T.reshape(-1) for r in res.results])
    loss = np.float32(np.mean(per_row.astype(np.float64)))
    return loss, cos
